# revision 1
# baseline (speedup 1.0000x reference)
"""Trainium2 Bass kernel for a dense transformer block (nn_Block_83880711291003).

Full (unsharded) inputs in, full output out. 8 NeuronCores:
  core c -> batch b = c//2, query-set p = c%2 (sequence-parallel within a pair).
Parity 0 owns q-regions {0,3} of four 512-token regions, parity 1 owns {1,2}
(balanced causal work: 1+4 == 2+3 kv-blocks). One SPMD program for all cores:
per-slot kv spans are padded to {1024, 2048} and causality enforced via
host-provided 0/1 masks, so no per-core control flow is needed.
"""

import sys

for _p in ("/opt/trn_rl_repo", "/root/.axon_site/_ro/trn_rl_repo"):
    if _p not in sys.path:
        sys.path.append(_p)

from contextlib import ExitStack

import ml_dtypes
import numpy as np

import concourse.bass as bass
import concourse.tile as tile
from concourse import mybir
from concourse.bass_utils import run_bass_kernel_spmd
from concourse.masks import make_identity
from concourse.vector_clock import ScopedClock

FP32 = mybir.dt.float32
BF16 = mybir.dt.bfloat16
BFNP = ml_dtypes.bfloat16

B, T, D = 4, 2048, 512
H, DK = 8, 64
NT = T // 128          # 16 token tiles of 128
OWN_T = T // 2         # 1024 own query tokens per core
OWN_NT = OWN_T // 128  # 8
FF = 4 * D             # 2048
EPS = 1e-5
SCALE = 1.0 / 8.0      # 1/sqrt(DK)
SPAN = (8, 16)         # kv span per slot, in 128-kv-tiles (padded, same all cores)
REGIONS = ((0, 3), (1, 2))  # q-region (512-token) assignment per core parity

# ---------------------------------------------------------------------------
# Workaround: this walrus build rejects >1 semaphore wait per instruction.
# ---------------------------------------------------------------------------
_uid = [0]


def _split_multi_waits(nc):
    for blk in nc.m.functions[0].blocks:
        insts = list(blk.instructions)
        out, changed = [], False
        for inst in insts:
            si = inst.sync_info
            waits = list(si.on_wait) if si else []
            if len(waits) > 1:
                changed = True
                for w in waits[:-1]:
                    _uid[0] += 1
                    nop = mybir.InstNoOp(name=f"I-waitfix-{_uid[0]}", ins=[], outs=[])
                    nop.engine = inst.engine
                    nop.sync_info = mybir.SyncInfo(on_wait=[w], on_update=[])
                    out.append(nop)
                inst.sync_info = mybir.SyncInfo(
                    on_wait=[waits[-1]], on_update=list(si.on_update)
                )
            out.append(inst)
        if changed:
            blk.instructions = out


def _patched_drain_and_barrier(self, tick_clock, wait_clock):
    nc = self.nc
    probe = nc.sync.nop()
    wait_clock.add_sem_waits(probe.ins, ScopedClock({None: tick_clock.global_clock}))
    nc.sync.drain()
    nc.all_engine_barrier()
    popped = nc._tile_sem_poison_stack.pop()
    assert popped is self._sem_poison
    nc.clear_and_free_semaphores(list(self.sems.allocated().values()))
    nc.all_engine_barrier()


tile.TileContext._drain_and_barrier = _patched_drain_and_barrier


# ---------------------------------------------------------------------------
# Device program (identical on all 8 cores)
# ---------------------------------------------------------------------------
def _build_program():
    nc = bass.Bass("TRN2", target_bir_lowering=False, debug=False)

    din = {}
    for name, shape, dt in [
        ("x_full", [T, D], FP32),
        ("x_own", [OWN_T, D], FP32),
        ("wq", [4, 128, D], BF16),
        ("wk", [4, 128, D], BF16),
        ("wv", [4, 128, D], BF16),
        ("wp", [H, 64, D], BF16),
        ("w1", [4, 128, FF], BF16),
        ("w2", [16, 128, D], BF16),
        ("bqk", [128, 8], FP32),      # cols 0-3: bq per pair; 4-7: bk per pair
        ("bv_row", [1, D], FP32),
        ("bp_row", [1, D], FP32),
        ("b2_row", [1, D], FP32),
        ("b1t", [128, 16], FP32),     # b1 reshaped (16,128).T
        ("masks", [16, 128, 512], BF16),
    ]:
        din[name] = nc.dram_tensor(name, shape, dt, kind="ExternalInput").ap()
    out_dram = nc.dram_tensor("out", [OWN_T, D], FP32, kind="ExternalOutput").ap()

    ACT = mybir.ActivationFunctionType

    with tile.TileContext(nc) as tc, ExitStack() as ctx:
        P = ctx.enter_context  # pool helper

        persist = P(tc.tile_pool(name="persist", bufs=1))
        wpool = P(tc.tile_pool(name="weights", bufs=1))
        xio = P(tc.tile_pool(name="xio", bufs=4))
        small = P(tc.tile_pool(name="small", bufs=4))
        hpool = P(tc.tile_pool(name="htok", bufs=4))
        ppool = P(tc.tile_pool(name="pT", bufs=8))
        opool = P(tc.tile_pool(name="outio", bufs=3))
        psA = P(tc.tile_pool(name="psA", bufs=4, space="PSUM"))   # matmul accumulators
        psO = P(tc.tile_pool(name="psO", bufs=2, space="PSUM"))   # [65,512]
        psP = P(tc.tile_pool(name="psP", bufs=2, space="PSUM"))   # proj + transposes

        # ---- prefetch the first x tiles ahead of all weight traffic ----
        x_pre = []
        for t in range(6):
            x_t = xio.tile([128, D], FP32, tag="xin", name="xin")
            nc.sync.dma_start(x_t[:], din["x_full"][t * 128:(t + 1) * 128, :])
            x_pre.append(x_t)

        # ---- small weights / constants (held for whole kernel) ----
        wq = [wpool.tile([128, D], BF16, tag=f"wq{c}", name=f"wq{c}") for c in range(4)]
        wk = [wpool.tile([128, D], BF16, tag=f"wk{c}", name=f"wk{c}") for c in range(4)]
        wv = [wpool.tile([128, D], BF16, tag=f"wv{c}", name=f"wv{c}") for c in range(4)]
        wp = [wpool.tile([64, D], BF16, tag=f"wp{c}", name=f"wp{c}") for c in range(H)]
        for c in range(4):
            nc.sync.dma_start(wq[c][:], din["wq"][c])
            nc.sync.dma_start(wk[c][:], din["wk"][c])
            nc.sync.dma_start(wv[c][:], din["wv"][c])
        for h in range(H):
            nc.sync.dma_start(wp[h][:], din["wp"][h])
        bqk = wpool.tile([128, 8], FP32, tag="bqk", name="bqk")
        nc.sync.dma_start(bqk[:], din["bqk"][:])
        b1t = wpool.tile([128, 16], FP32, tag="b1t", name="b1t")
        nc.sync.dma_start(b1t[:], din["b1t"][:])

        def bcast_row(name):
            t = wpool.tile([128, D], FP32, tag=f"bc_{name}", name=f"bc_{name}")
            src = din[name]
            ap = bass.AP(tensor=src.tensor, offset=src.offset,
                         ap=[[0, 128], src.ap[1]])
            nc.gpsimd.dma_start(out=t[:], in_=ap)
            return t

        bv_b = bcast_row("bv_row")
        bp_b = bcast_row("bp_row")
        b2_b = bcast_row("b2_row")

        mp_ctx = ExitStack()
        mpool = mp_ctx.enter_context(tc.tile_pool(name="mpool", bufs=1))
        mask_t = mpool.tile([128, 16, 512], BF16, tag="masks", name="masks")
        nc.gpsimd.dma_start(mask_t[:], din["masks"].rearrange("w p f -> p w f"))

        ident = wpool.tile([128, 128], BF16, tag="ident", name="ident")
        make_identity(nc, ident[:])
        ident32 = wpool.tile([8, 8], FP32, tag="ident32", name="ident32")
        make_identity(nc, ident32[:])

        eps_t = wpool.tile([128, 1], FP32, tag="eps", name="eps")
        nc.vector.memset(eps_t[:], EPS)

        # ---- activations that live until the end of attention/projection ----
        kT = [persist.tile([128, T], BF16, tag=f"kT{pr}", name=f"kT{pr}") for pr in range(4)]
        qT = [persist.tile([128, OWN_T], BF16, tag=f"qT{pr}", name=f"qT{pr}") for pr in range(4)]
        v1 = persist.tile([128, NT, H, 65], BF16, tag="v1", name="v1")
        oTu = [persist.tile([64, OWN_T], BF16, tag=f"oTu{h}", name=f"oTu{h}") for h in range(H)]
        # softmax denominators: row = head, col = slot*512 + q (own order)
        den = persist.tile([8, OWN_T], FP32, tag="den", name="den")
        x2 = [persist.tile([128, D], FP32, tag=f"x2_{t}", name=f"x2_{t}") for t in range(OWN_NT)]
        recip = [persist.tile([128, 8], FP32, tag=f"rc{t}", name=f"rc{t}") for t in range(OWN_NT)]
        h2T = [persist.tile([128, OWN_T], BF16, tag=f"h2T{c}", name=f"h2T{c}")
               for c in range(4)]

        def layer_norm_tile(x_t, h_out, apply_eng="dve"):
            """h_out(bf16) = (x_t - mean)/sqrt(var+eps), per token row."""
            stats = small.tile([128, 6], FP32, tag="bnst", name="bnst")
            nc.vector.bn_stats(out=stats[:], in_=x_t[:])
            mv = small.tile([128, 2], FP32, tag="bnmv", name="bnmv")
            nc.vector.bn_aggr(out=mv[:], in_=stats[:])
            rs = small.tile([128, 1], FP32, tag="rs", name="rs")
            nc.scalar.activation(out=rs[:], in_=mv[:, 1:2], func=ACT.Sqrt,
                                 bias=eps_t[:], scale=1.0)
            nc.vector.reciprocal(out=rs[:], in_=rs[:])
            if apply_eng == "dve":
                nc.vector.tensor_scalar(
                    out=h_out[:], in0=x_t[:], scalar1=mv[:, 0:1],
                    scalar2=rs[:], op0=mybir.AluOpType.subtract,
                    op1=mybir.AluOpType.mult)
            else:
                nmr = small.tile([128, 1], FP32, tag="nmr", name="nmr")
                nc.vector.scalar_tensor_tensor(
                    out=nmr[:], in0=mv[:, 0:1], scalar=-1.0, in1=rs[:],
                    op0=mybir.AluOpType.mult, op1=mybir.AluOpType.mult)
                nc.scalar.activation(out=h_out[:], in_=x_t[:], func=ACT.Identity,
                                     bias=nmr[:], scale=rs[:])

        def transpose128(dst, dst_col, src, src_col, copy_eng="act", alt=False):
            """dst[:, dst_col:dst_col+128] (bf16) = src[:, src_col:+128].T"""
            c = src_col // 128
            if alt and c % 2 == 1:
                ps = psO.tile([128, 128], BF16, tag="psO", name="tp")
            else:
                ps = psP.tile([128, 128], BF16, tag="psP", name="tp")
            nc.tensor.transpose(ps[:], src[:, src_col:src_col + 128], ident[:])
            if copy_eng == "act":
                nc.scalar.copy(dst[:, dst_col:dst_col + 128], ps[:])
            else:
                nc.vector.tensor_copy(dst[:, dst_col:dst_col + 128], ps[:])

        # ---- attention slot s + its projection/LN2 epilogue ----
        def att_slot(s):
            span = SPAN[s]
            for h in range(H):
                pr, sub = h // 2, h % 2
                krows = kT[pr][sub * 64:(sub + 1) * 64, :]
                qrows = qT[pr][sub * 64:(sub + 1) * 64, :]
                ops = psO.tile([65, 512], FP32, tag="psO", name="psO")
                for j in range(span):
                    sps = psA.tile([128, 512], FP32, tag="ps512", name="ps512")
                    nc.tensor.matmul(sps[:],
                                     krows[:, j * 128:(j + 1) * 128],
                                     qrows[:, s * 512:(s + 1) * 512],
                                     start=True, stop=True)
                    pT = ppool.tile([128, 512], BF16, tag="pT", name="pT")
                    nc.scalar.activation(out=pT[:], in_=sps[:], func=ACT.Exp,
                                         scale=SCALE)
                    in_window = (s == 0 and j < 8) or (s == 1 and j >= 8)
                    if in_window:
                        eng = nc.vector if j % 2 == 0 else nc.gpsimd
                        eng.tensor_mul(pT[:], pT[:], mask_t[:, j, :])
                    nc.tensor.matmul(ops[:], v1[:, j, h, :], pT[:],
                                     start=(j == 0), stop=(j == span - 1))
                drow = small.tile([1, 512], FP32, tag="drow", name="drow")
                nc.vector.tensor_copy(drow[:], ops[64:65, :])
                nc.sync.dma_start(den[h:h + 1, s * 512:(s + 1) * 512], drow[:])
                nc.vector.tensor_copy(
                    oTu[h][:, s * 512:(s + 1) * 512], ops[0:64, :])

            for t in range(s * 4, s * 4 + 4):
                ps = psO.tile([128, 8], FP32, tag="psO", name="tp32")
                nc.tensor.transpose(ps[:], den[0:8, t * 128:(t + 1) * 128],
                                    ident32[0:8, 0:8])
                nc.vector.reciprocal(out=recip[t][:], in_=ps[:])

                x_t = xio.tile([128, D], FP32, tag="xin", name="xin")
                nc.sync.dma_start(x_t[:],
                                  din["x_own"][t * 128:(t + 1) * 128, :])
                xb = xio.tile([128, D], FP32, tag="xb", name="xb")
                nc.gpsimd.tensor_add(xb[:], x_t[:], bp_b[:])
                for h in range(H):
                    ps = psP.tile([128, 512], FP32, tag="psP", name="psP")
                    nc.tensor.matmul(ps[:], oTu[h][:, t * 128:(t + 1) * 128],
                                     wp[h][:], start=True, stop=True)
                    src2 = xb if h == 0 else x2[t]
                    nc.vector.scalar_tensor_tensor(
                        out=x2[t][:], in0=ps[:], scalar=recip[t][:, h:h + 1],
                        in1=src2[:], op0=mybir.AluOpType.mult,
                        op1=mybir.AluOpType.add)

                h_t = hpool.tile([128, D], BF16, tag="h1", name="h1")
                layer_norm_tile(x2[t], h_t, apply_eng="act" if s == 0 else "dve")
                for c in range(4):
                    transpose128(h2T[c], t * 128, h_t, c * 128,
                                 "dve" if s == 0 else "act")


        with ExitStack() as phaseA:
            hTpool = phaseA.enter_context(tc.tile_pool(name="hTpool", bufs=1))
            hT = [hTpool.tile([128, T], BF16, tag=f"hT{c}", name=f"hT{c}")
                  for c in range(4)]
            hTo = [hTpool.tile([128, OWN_T], BF16, tag=f"hTo{c}", name=f"hTo{c}")
                   for c in range(4)]

            # ---- stage 1+2+3 interleaved: LN1 (full + own merged), V per
            # tile, K/Q per 512-chunk as soon as its 4 tiles are ready ----
            nc.vector.memset(v1[:, :, :, 64], 1.0)

            def ln_own_tile(t):
                x_t = xio.tile([128, D], FP32, tag="xin", name="xin")
                nc.sync.dma_start(x_t[:], din["x_own"][t * 128:(t + 1) * 128, :])
                h_t = hpool.tile([128, D], BF16, tag="h1", name="h1")
                layer_norm_tile(x_t, h_t)
                for c in range(4):
                    transpose128(hTo[c], t * 128, h_t, c * 128, "act")
                if t % 4 == 3:
                    tc5 = t // 4
                    for pr in range(4):
                        ps = psA.tile([128, 512], FP32, tag="ps512", name="ps512")
                        for c in range(4):
                            nc.tensor.matmul(
                                ps[:], wq[c][:, pr * 128:(pr + 1) * 128],
                                hTo[c][:, tc5 * 512:(tc5 + 1) * 512],
                                start=(c == 0), stop=(c == 3))
                        nc.scalar.activation(
                            out=qT[pr][:, tc5 * 512:(tc5 + 1) * 512], in_=ps[:],
                            func=ACT.Identity, bias=bqk[:, pr:pr + 1],
                            scale=1.0)

            def full_tile(t):
                if t < len(x_pre):
                    x_t = x_pre[t]
                else:
                    x_t = xio.tile([128, D], FP32, tag="xin", name="xin")
                    nc.sync.dma_start(x_t[:],
                                      din["x_full"][t * 128:(t + 1) * 128, :])
                h_t = hpool.tile([128, D], BF16, tag="h1", name="h1")
                layer_norm_tile(x_t, h_t)
                for c in range(4):
                    transpose128(hT[c], t * 128, h_t, c * 128, "act")
                ps = psA.tile([128, 512], FP32, tag="ps512", name="ps512")
                for c in range(4):
                    nc.tensor.matmul(ps[:], hT[c][:, t * 128:(t + 1) * 128],
                                     wv[c][:], start=(c == 0), stop=(c == 3))
                nc.vector.tensor_add(
                    v1[:, t, :, 0:64],
                    ps[:].rearrange("p (h k) -> p h k", h=H),
                    bv_b[:].rearrange("p (h k) -> p h k", h=H))
                if t % 4 == 3:
                    tc5 = t // 4
                    for pr in range(4):
                        ps = psA.tile([128, 512], FP32, tag="ps512", name="ps512")
                        for c in range(4):
                            nc.tensor.matmul(
                                ps[:], wk[c][:, pr * 128:(pr + 1) * 128],
                                hT[c][:, tc5 * 512:(tc5 + 1) * 512],
                                start=(c == 0), stop=(c == 3))
                        nc.scalar.activation(
                            out=kT[pr][:, tc5 * 512:(tc5 + 1) * 512], in_=ps[:],
                            func=ACT.Identity, bias=bqk[:, 4 + pr:5 + pr],
                            scale=1.0)

            for t in range(8):
                full_tile(t)
            for t in range(4):
                ln_own_tile(t)
            for t in range(8, 16):
                full_tile(t)
            for t in range(4, 8):
                ln_own_tile(t)
            att_slot(0)

        att_slot(1)
        mp_ctx.close()

        # ---- stage 8: FFN (weights + hidden live only here) ----
        with ExitStack() as phaseC:
            fpool = phaseC.enter_context(tc.tile_pool(name="fpool", bufs=1))
            f1pool = phaseC.enter_context(tc.tile_pool(name="f1pool", bufs=18))
            w1 = [fpool.tile([128, FF], BF16, tag=f"w1{c}", name=f"w1{c}")
                  for c in range(4)]
            w2 = [fpool.tile([128, D], BF16, tag=f"w2{c}", name=f"w2{c}")
                  for c in range(16)]
            for c in range(4):
                nc.sync.dma_start(w1[c][:], din["w1"][c])
            for c in range(16):
                nc.sync.dma_start(w2[c][:], din["w2"][c])

            # FFN, one 512-token chunk at a time
            for tc5 in range(2):
                f1 = []
                for ht in range(16):
                    ps = psA.tile([128, 512], FP32, tag="ps512", name="ps512")
                    for c in range(4):
                        nc.tensor.matmul(ps[:], w1[c][:, ht * 128:(ht + 1) * 128],
                                         h2T[c][:, tc5 * 512:(tc5 + 1) * 512],
                                         start=(c == 0), stop=(c == 3))
                    f1t = f1pool.tile([128, 512], BF16, tag="f1", name="f1")
                    nc.scalar.activation(out=f1t[:], in_=ps[:], func=ACT.Relu,
                                         bias=b1t[:, ht:ht + 1], scale=1.0)
                    f1.append(f1t)

                for tt in range(4):
                    t = tc5 * 4 + tt
                    psa = psA.tile([128, 512], FP32, tag="ps512", name="ps512")
                    for c in range(8):
                        nc.tensor.matmul(psa[:], f1[c][:, tt * 128:(tt + 1) * 128],
                                         w2[c][:], start=(c == 0), stop=(c == 7))
                    psb = psO.tile([128, 512], FP32, tag="psO", name="psb")
                    for c in range(8, 16):
                        nc.tensor.matmul(psb[:], f1[c][:, tt * 128:(tt + 1) * 128],
                                         w2[c][:], start=(c == 8), stop=(c == 15))
                    o_t = opool.tile([128, D], FP32, tag="ot", name="ot")
                    nc.vector.tensor_add(o_t[:], psa[:], x2[t][:])
                    nc.vector.tensor_add(o_t[:], o_t[:], psb[:])
                    nc.gpsimd.tensor_add(o_t[:], o_t[:], b2_b[:])
                    nc.sync.dma_start(out_dram[t * 128:(t + 1) * 128, :], o_t[:])

    _split_multi_waits(nc)
    return nc


_NC_CACHE = None


def _get_nc():
    global _NC_CACHE
    if _NC_CACHE is None:
        _NC_CACHE = _build_program()
    return _NC_CACHE


# ---------------------------------------------------------------------------
# Host side
# ---------------------------------------------------------------------------
def _fold_weights(Wq, bq, Wk, bk, Wv, bv, Wp, bp, W1, b1, W2, b2, g1, be1,
                  g2, be2):
    f64 = np.float64
    # LN1 gain/shift folded into per-head QKV projections.
    Wq_e = (g1.astype(f64)[None, :, None] * Wq.astype(f64))      # [H,D,DK]
    Wk_e = (g1.astype(f64)[None, :, None] * Wk.astype(f64))
    Wv_e = (g1.astype(f64)[None, :, None] * Wv.astype(f64))
    bq_e = bq.astype(f64) + np.einsum("d,hdk->hk", be1.astype(f64), Wq.astype(f64))
    bk_e = bk.astype(f64) + np.einsum("d,hdk->hk", be1.astype(f64), Wk.astype(f64))
    bv_e = bv.astype(f64) + np.einsum("d,hdk->hk", be1.astype(f64), Wv.astype(f64))
    W1_e = g2.astype(f64)[:, None] * W1.astype(f64)
    b1_e = b1.astype(f64) + be2.astype(f64) @ W1.astype(f64)

    def head_major(W):  # [H,D,DK] -> [D, H*DK] -> [4,128,512]
        return np.transpose(W, (1, 0, 2)).reshape(D, H * DK).reshape(4, 128, H * DK)

    out = {}
    out["wq"] = head_major(Wq_e).astype(BFNP)
    out["wk"] = head_major(Wk_e).astype(BFNP)
    out["wv"] = head_major(Wv_e).astype(BFNP)
    out["wp"] = Wp.astype(f64).reshape(H, 64, D).astype(BFNP)
    out["w1"] = W1_e.reshape(4, 128, FF).astype(BFNP)
    out["w2"] = W2.astype(f64).reshape(16, 128, D).astype(BFNP)
    bqk = np.zeros((128, 8), np.float32)
    for pr in range(4):
        bqk[:, pr] = np.concatenate([bq_e[2 * pr], bq_e[2 * pr + 1]])
        bqk[:, 4 + pr] = np.concatenate([bk_e[2 * pr], bk_e[2 * pr + 1]])
    out["bqk"] = bqk
    out["bv_row"] = bv_e.reshape(1, H * DK).astype(np.float32)
    out["bp_row"] = bp.reshape(1, D).astype(np.float32)
    out["b2_row"] = b2.reshape(1, D).astype(np.float32)
    out["b1t"] = np.ascontiguousarray(
        b1_e.reshape(16, 128).T).astype(np.float32)
    return out


def _build_masks(p):
    """[16,128,512] bf16: slot0 window = kv-tiles 0..7, slot1 = kv-tiles 8..15."""
    r0, r1 = REGIONS[p]
    masks = np.zeros((16, 128, 512), np.float32)
    q0 = np.arange(512)
    for w in range(16):
        r = r0 if w < 8 else r1
        kv = w * 128 + np.arange(128)
        qg = r * 512 + q0
        masks[w] = (kv[:, None] <= qg[None, :]).astype(np.float32)
    return masks.astype(BFNP)


def kernel(x, Wq, bq, Wk, bk, Wv, bv, Wp, bp, W1, b1, W2, b2, g1, be1, g2, be2):
    x = np.asarray(x, np.float32)
    folded = _fold_weights(
        np.asarray(Wq), np.asarray(bq), np.asarray(Wk), np.asarray(bk),
        np.asarray(Wv), np.asarray(bv), np.asarray(Wp), np.asarray(bp),
        np.asarray(W1), np.asarray(b1), np.asarray(W2), np.asarray(b2),
        np.asarray(g1), np.asarray(be1), np.asarray(g2), np.asarray(be2))

    masks_by_p = [_build_masks(0), _build_masks(1)]
    in_maps = []
    for c in range(8):
        b, p = c // 2, c % 2
        r0, r1 = REGIONS[p]
        x_own = np.concatenate(
            [x[b, r0 * 512:(r0 + 1) * 512], x[b, r1 * 512:(r1 + 1) * 512]])
        m = dict(folded)
        m["x_full"] = np.ascontiguousarray(x[b])
        m["x_own"] = np.ascontiguousarray(x_own)
        m["masks"] = masks_by_p[p]
        in_maps.append(m)

    nc = _get_nc()
    res = run_bass_kernel_spmd(nc, in_maps, list(range(8)))

    out = np.empty((B, T, D), np.float32)
    for c in range(8):
        b, p = c // 2, c % 2
        r0, r1 = REGIONS[p]
        o = res.results[c]["out"]
        out[b, r0 * 512:(r0 + 1) * 512] = o[:512]
        out[b, r1 * 512:(r1 + 1) * 512] = o[512:]
    return out

